# revision 1
# baseline (speedup 1.0000x reference)
"""Trainium2 Bass kernel for MembraneNet (PINN forward + analytic PDE residual).

Math: the reference computes, per collocation point p=(x,y):
  u(p)  = Wout . h3 + bout           (4-layer tanh MLP, H=64)
  PDE   = K*(uxx+uyy) + Kx*ux + Ky*uy + f
The reference builds per-point (H,H) Jacobian chain products; here we use
forward-mode propagation of (h, dh/dx, dh/dy, lap h) per layer which is
O(H^2) per point instead of O(H^3):
  z  = W h + b ;  h' = tanh(z) ;  d = 1-h'^2 ;  s = -2 h' d
  zx = W gx    ;  gx' = d . zx
  zy = W gy    ;  gy' = d . zy
  zl = W lap   ;  lap' = s . (zx^2 + zy^2) + d . zl
Then u = Wout.h3+bout, ux = Wout.gx3, uy = Wout.gy3, uxx+uyy = Wout.lap3.

Sharding: batch (16384) split over 8 cores (2048/core). On each core the 2048
points are stacked as 2 chunks of 1024 on SBUF partitions 0-63 / 64-127 with
block-diagonal replicated weights, so matmuls use the full 128-wide PE array.
gx is propagated with a flipped sign (one fewer op at layer 0); squares are
unaffected and the final PDE assembly subtracts the Kx/Ky terms instead.
lap is carried unsummed as [dd, ee]; the next layer's zl matmul accumulates
both in PSUM, trading a cheap extra PE pass for a DVE add on the critical path.
"""

import sys

sys.path.insert(0, "/opt/trn_rl_repo")

import numpy as np
from contextlib import ExitStack

import concourse.bass as bass
import concourse.mybir as mybir
import concourse.tile as tile
from concourse.masks import make_identity

B = 16384
H = 64
L = 4
NCORES = 8
BC = B // NCORES          # 2048 points per core
F = BC // 2               # 1024 free-dim columns (2 chunks stacked)
NH = 512                  # matmul free-dim per instruction (1 PSUM bank fp32)
NMM = F // NH             # matmul instructions per pass
FT = BC // 128            # 16: free dim of final per-point [128, FT] tiles

f32 = mybir.dt.float32
AF = mybir.ActivationFunctionType
OP = mybir.AluOpType

# dtype knobs: matmul operand dtype and elementwise dtype
MM_DT = mybir.dt.float32r  # f32r: 1 cyc/row on PE vs 4 for f32
EW_DT = f32


def _col(ap):
    """[64] dram vector -> [64,1] view for column DMA."""
    return ap[:, None]


def _patch_walrus_ldw_opt():
    """Re-enable walrus LDWEIGHTS dedup (concourse pins it off)."""
    from concourse import bass_utils as _bu

    if getattr(_bu, "_ldw_opt_patched", False):
        return
    orig = _bu.run_command

    def patched(argv, **kw):
        argv = [
            "--enable-ldw-opt=true" if a == "--enable-ldw-opt=false" else a
            for a in argv
        ]
        return orig(argv, **kw)

    _bu.run_command = patched
    _bu._ldw_opt_patched = True


def _legalize_sync_waits(bj: bytes) -> bytes:
    """The walrus in this container accepts at most ONE on_wait per
    instruction, but Tile emits several. Move excess waits into standalone
    EventSemaphore instructions right before the owner (same engine, so the
    sequencer executes them first) — the exact encoding raw-bass wait_ge uses.
    """
    import json

    m = json.loads(bj)
    n = 0
    for fn in m.get("functions", []):
        for blk in fn.get("blocks", []):
            out = []
            for ins in blk.get("instructions", []):
                si = ins.get("sync_info") or {}
                waits = si.get("on_wait") or []
                if len(waits) > 1:
                    for w in waits[:-1]:
                        n += 1
                        out.append(
                            {
                                "name": f"lsw_{n}",
                                "opcode": "EventSemaphore",
                                "engine": ins["engine"],
                                "ins": [],
                                "outs": [],
                                "debug": ins.get("debug", 0),
                                "sync_info": {"on_update": [], "on_wait": [w]},
                            }
                        )
                    si["on_wait"] = waits[-1:]
                out.append(ins)
            blk["instructions"] = out
    return json.dumps(m).encode()


def build_nc(mm_dt=MM_DT, ew_dt=EW_DT):
    nc = bass.Bass()

    # ---- I/O ----
    xy_d = nc.dram_tensor("xy", [BC, 2], f32, kind="ExternalInput")
    K_d = nc.dram_tensor("K", [BC], f32, kind="ExternalInput")
    Kx_d = nc.dram_tensor("Kx", [BC], f32, kind="ExternalInput")
    Ky_d = nc.dram_tensor("Ky", [BC], f32, kind="ExternalInput")
    f_d = nc.dram_tensor("f", [BC], f32, kind="ExternalInput")
    W_d = [nc.dram_tensor("W0", [H, 2], f32, kind="ExternalInput")]
    b_d = [nc.dram_tensor("b0", [H], f32, kind="ExternalInput")]
    for i in range(1, L):
        W_d.append(nc.dram_tensor(f"W{i}", [H, H], f32, kind="ExternalInput"))
        b_d.append(nc.dram_tensor(f"b{i}", [H], f32, kind="ExternalInput"))
    Wout_d = nc.dram_tensor("Wout", [H], f32, kind="ExternalInput")
    bout_d = nc.dram_tensor("bout", [1], f32, kind="ExternalInput")
    u_d = nc.dram_tensor("u", [BC], f32, kind="ExternalOutput")
    pde_d = nc.dram_tensor("pde", [BC], f32, kind="ExternalOutput")

    with tile.TileContext(nc) as tc, ExitStack() as ctx:
        const = ctx.enter_context(tc.tile_pool(name="const", bufs=1))
        sb = ctx.enter_context(tc.tile_pool(name="sb", bufs=2))
        ps = ctx.enter_context(tc.tile_pool(name="ps", bufs=1, space="PSUM"))

        # ---- early DMAs (contiguous only; strided patterns are handled
        # on-chip via PE transpose / strided engine copies) ----
        # xy pairs, one partition per chunk
        xystg = sb.tile([128, 2 * F], f32, tag="xystg")
        nc.sync.dma_start(out=xystg[0:1, :], in_=xy_d[0:F, :])
        nc.scalar.dma_start(out=xystg[H : H + 1, :], in_=xy_d[F:BC, :])

        # W0 for both chunks (contiguous [64,2])
        w0c = const.tile([128, 2], f32, tag="w0c")
        nc.sync.dma_start(out=w0c[0:H, :], in_=W_d[0][:, :])
        nc.scalar.dma_start(out=w0c[H:128, :], in_=W_d[0][:, :])

        # bias/Wout row staging for the column transpose: row k = [b_k|b_k],
        # row 4 = [Wout|Wout]
        brows = sb.tile([128, 128], f32, tag="brows")
        for k in range(L):
            nc.sync.dma_start(out=brows[k : k + 1, 0:H], in_=b_d[k][None, :])
            nc.scalar.dma_start(out=brows[k : k + 1, H:128], in_=b_d[k][None, :])
        nc.sync.dma_start(out=brows[4:5, 0:H], in_=Wout_d[None, :])
        nc.scalar.dma_start(out=brows[4:5, H:128], in_=Wout_d[None, :])

        ident = const.tile([128, 128], f32, tag="ident")
        make_identity(nc, ident[:])

        # shared f32 zeros (memset cannot target f32r tiles)
        zF = const.tile([128, F], f32, tag="zF")
        nc.gpsimd.memset(zF[:], 0.0)

        # ---- weight prep: natural loads + PE transpose ----
        # block-diag(W, W) staged naturally, PE-transposed to block-diag(WT,WT).
        # Layer 0 (K=2): x/y input rows must sit on 32-aligned partitions, so
        # W0's x column goes to staging col 0 / y to col 32 (chunk B: 64/96),
        # making the transposed lhsT contract rhs partitions {0,32,64,96}.
        WT = []
        for k in range(L):
            wnat = sb.tile([128, 128], f32, tag="wnat")
            nc.gpsimd.memset(wnat[:], 0.0)
            if k == 0:
                nc.vector.tensor_copy(wnat[0:H, 0:1], w0c[0:H, 0:1])
                nc.vector.tensor_copy(wnat[0:H, 32:33], w0c[0:H, 1:2])
                nc.vector.tensor_copy(wnat[H:128, H : H + 1], w0c[H:128, 0:1])
                nc.vector.tensor_copy(
                    wnat[H:128, H + 32 : H + 33], w0c[H:128, 1:2]
                )
            else:
                nc.sync.dma_start(out=wnat[0:H, 0:H], in_=W_d[k][:, :])
                nc.scalar.dma_start(out=wnat[H:128, H:128], in_=W_d[k][:, :])
            wt_ps = ps.tile([128, 128], f32, tag="zlp")
            nc.tensor.transpose(wt_ps[:], wnat[:], ident[:])
            wt = const.tile([128, 128], mm_dt, tag=f"wt{k}")
            nc.scalar.copy(wt[:], wt_ps[:])
            WT.append(wt)

        # bias/Wout columns via one transpose: col k = [b_k;b_k], col4 = Wout
        bw_ps = ps.tile([128, 128], f32, tag="zyp")
        nc.tensor.transpose(bw_ps[:], brows[:], ident[:])
        bwc = const.tile([128, 8], f32, tag="bwc")
        nc.vector.tensor_copy(bwc[:], bw_ps[:, 0:8])
        bcol = [bwc[:, k : k + 1] for k in range(L)]
        wcol = bwc[:, 4:5]

        # output reduction lhsTs: one [128, 32] tile; 8-col group q has
        # local col 2q = [Wout;0], 2q+1 = [0;Wout]
        wl32f = sb.tile([128, 32], f32, tag="wl32f")
        nc.vector.memset(wl32f[:], 0.0)
        for q in range(4):
            nc.vector.tensor_copy(wl32f[0:H, 10 * q : 10 * q + 1], wcol[0:H])
            nc.vector.tensor_copy(
                wl32f[H:128, 10 * q + 1 : 10 * q + 2], wcol[H:128]
            )
        wl32 = const.tile([128, 32], mm_dt, tag="wl32")
        nc.vector.tensor_copy(wl32[:], wl32f[:])
        wout_l = [wl32[:, 8 * q : 8 * q + 8] for q in range(4)]

        # q0p1 = W0x^2 + W0y^2 per partition (layer-0 laplacian source)
        w0sq = const.tile([128, 2], f32, tag="w0sq")
        nc.vector.tensor_mul(w0sq[:], w0c[:], w0c[:])
        q0p1 = const.tile([128, 1], f32, tag="q0p1")
        nc.vector.tensor_reduce(
            out=q0p1[:], in_=w0sq[:], op=OP.add, axis=mybir.AxisListType.X
        )

        # bout broadcast column
        bout_c = const.tile([128, 1], f32, tag="bout_c")
        bout_bcast = bass.AP(
            tensor=bout_d[:].tensor, offset=0, ap=[[0, 128], [0, 1]]
        )
        nc.gpsimd.dma_start(out=bout_c[:], in_=bout_bcast)

        # per-point final tiles of K, Kx, Ky, f: [128, FT]
        kq = {}
        for name, d in (("K", K_d), ("Kx", Kx_d), ("Ky", Ky_d), ("f", f_d)):
            t = const.tile([128, FT], f32, tag=f"kq_{name}")
            nc.gpsimd.dma_start(
                out=t[:], in_=d[:].rearrange("(p j) -> p j", p=128)
            )
            kq[name] = t

        # ---- layer 0 input: deinterleave xy into feature-major rows ----
        # xyT rows 0,32 = x,y of chunk A; rows 64,96 = chunk B (32-aligned)
        xyT = sb.tile([128, F], mm_dt, tag="xyT")
        nc.vector.tensor_copy(xyT[:], zF[:])
        xyA = xystg[0:1, :].rearrange("p (b t) -> p b t", t=2)
        xyB = xystg[H : H + 1, :].rearrange("p (b t) -> p b t", t=2)
        nc.vector.tensor_copy(xyT[0:1, :], xyA[:, :, 0])
        nc.scalar.copy(xyT[32:33, :], xyA[:, :, 1])
        nc.vector.tensor_copy(xyT[H : H + 1, :], xyB[:, :, 0])
        nc.scalar.copy(xyT[96:97, :], xyB[:, :, 1])

        def mm_pass(out_ps, lhsT, rhs, start=True, stop=True):
            for j in range(NMM):
                s = slice(j * NH, (j + 1) * NH)
                nc.tensor.matmul(
                    out_ps[:, s], lhsT[:], rhs[:, s], start=start, stop=stop
                )

        # ---- layer 0 ----
        h = sb.tile([128, F], mm_dt, tag="h")
        hsq = sb.tile([128, F], ew_dt, tag="hsq")
        dbar = sb.tile([128, F], ew_dt, tag="dbar")
        gx = sb.tile([128, F], mm_dt, tag="gx")
        gy = sb.tile([128, F], mm_dt, tag="gy")
        m2 = sb.tile([128, F], ew_dt, tag="m2")
        lap0 = sb.tile([128, F], mm_dt, tag="dd")

        zp = ps.tile([128, F], f32, tag="zp")
        mm_pass(zp, WT[0], xyT)
        nc.scalar.activation(h[:], zp[:], AF.Tanh, bias=bcol[0], scale=1.0)
        nc.scalar.activation(hsq[:], h[:], AF.Square)
        nc.vector.tensor_scalar_add(dbar[:], hsq[:], -1.0)          # h^2-1 = -d
        # gx stored with flipped sign: gx = dbar*W0x = -(d*W0x)
        nc.vector.tensor_scalar_mul(gx[:], dbar[:], w0c[:, 0:1])
        nc.vector.tensor_scalar_mul(gy[:], dbar[:], w0c[:, 1:2])
        nc.vector.scalar_tensor_tensor(
            m2[:], h[:], 2.0, dbar[:], OP.mult, OP.mult
        )  # 2 h dbar = -2 h d = s/q-part
        nc.vector.tensor_scalar_mul(lap0[:], m2[:], q0p1[:])        # s * q0
        lsrc = [lap0]

        # ---- layers 1..3 ----
        for k in range(1, L):
            zp = ps.tile([128, F], f32, tag="zp")
            zxp = ps.tile([128, F], f32, tag="zxp")
            zyp = ps.tile([128, F], f32, tag="zyp")
            zlp = ps.tile([128, F], f32, tag="zlp")
            mm_pass(zp, WT[k], h)
            mm_pass(zxp, WT[k], gx)
            mm_pass(zyp, WT[k], gy)
            for i, ls in enumerate(lsrc):
                mm_pass(zlp, WT[k], ls, start=(i == 0), stop=(i == len(lsrc) - 1))

            h = sb.tile([128, F], mm_dt, tag="h")
            hsq = sb.tile([128, F], ew_dt, tag="hsq")
            dbar = sb.tile([128, F], ew_dt, tag="dbar")
            m2 = sb.tile([128, F], ew_dt, tag="m2")
            nc.scalar.activation(h[:], zp[:], AF.Tanh, bias=bcol[k], scale=1.0)
            nc.scalar.activation(hsq[:], h[:], AF.Square)
            nc.vector.tensor_scalar_add(dbar[:], hsq[:], -1.0)
            nc.vector.scalar_tensor_tensor(
                m2[:], h[:], 2.0, dbar[:], OP.mult, OP.mult
            )  # = s (off critical lap path)

            zxsq = sb.tile([128, F], ew_dt, tag="zxsq")
            zysq = sb.tile([128, F], ew_dt, tag="zysq")
            nc.scalar.activation(zxsq[:], zxp[:], AF.Square)
            nc.scalar.activation(zysq[:], zyp[:], AF.Square)

            gx = sb.tile([128, F], mm_dt, tag="gx")
            gy = sb.tile([128, F], mm_dt, tag="gy")
            nc.vector.scalar_tensor_tensor(
                gx[:], zxp[:], -1.0, dbar[:], OP.mult, OP.mult
            )  # zx*d
            nc.vector.scalar_tensor_tensor(
                gy[:], zyp[:], -1.0, dbar[:], OP.mult, OP.mult
            )

            q = sb.tile([128, F], ew_dt, tag="q")
            dd = sb.tile([128, F], mm_dt, tag="dd")
            ee = sb.tile([128, F], mm_dt, tag="ee")
            nc.vector.tensor_add(q[:], zxsq[:], zysq[:])
            nc.vector.tensor_mul(dd[:], q[:], m2[:])               # s*q
            nc.vector.scalar_tensor_tensor(
                ee[:], zlp[:], -1.0, dbar[:], OP.mult, OP.mult
            )  # d*zl
            lsrc = [dd, ee]

        # ---- output reductions: two psum tiles so u/ux/uy post-processing
        # overlaps the lap-path tail ----
        rp1 = ps.tile([8, F], f32, tag="zp")
        srcs1 = [(0, h), (1, gx), (2, gy)]
        for j in range(NMM):
            s = slice(j * NH, (j + 1) * NH)
            for i, (q_, src) in enumerate(srcs1):
                nc.tensor.matmul(
                    rp1[:, s], wout_l[q_], src[:, s],
                    start=(i == 0), stop=(i == len(srcs1) - 1),
                )
        rp2 = ps.tile([8, F], f32, tag="zyp")
        for j in range(NMM):
            s = slice(j * NH, (j + 1) * NH)
            for i, ls in enumerate(lsrc):
                nc.tensor.matmul(
                    rp2[:, s], wout_l[0], ls[:, s],
                    start=(i == 0), stop=(i == len(lsrc) - 1),
                )
        red1 = sb.tile([6, F], f32, tag="red1")
        red2 = sb.tile([2, F], f32, tag="red2")
        nc.vector.tensor_copy(red1[:], rp1[0:6, :])
        nc.vector.tensor_copy(red2[:], rp2[0:2, :])

        # ---- reshape rows -> [128, FT] per-point tiles ----
        fin = {}
        for q_, name in enumerate(("u", "ux", "uy")):
            t = sb.tile([128, FT], f32, tag=f"fin_{name}")
            nc.sync.dma_start(out=t[0:H, :], in_=red1[2 * q_ : 2 * q_ + 1, :])
            nc.scalar.dma_start(
                out=t[H:128, :], in_=red1[2 * q_ + 1 : 2 * q_ + 2, :]
            )
            fin[name] = t
        t = sb.tile([128, FT], f32, tag="fin_S")
        nc.sync.dma_start(out=t[0:H, :], in_=red2[0:1, :])
        nc.scalar.dma_start(out=t[H:128, :], in_=red2[1:2, :])
        fin["S"] = t

        # ---- final assembly ----
        u_fin = sb.tile([128, FT], f32, tag="u_fin")
        nc.vector.tensor_scalar_add(u_fin[:], fin["u"][:], bout_c[:])
        nc.sync.dma_start(out=u_d[:].rearrange("(p j) -> p j", p=128), in_=u_fin[:])

        t1 = sb.tile([128, FT], f32, tag="t1")
        t2 = sb.tile([128, FT], f32, tag="t2")
        pde = sb.tile([128, FT], f32, tag="pde")
        nc.vector.tensor_mul(t1[:], kq["Kx"][:], fin["ux"][:])
        nc.vector.tensor_sub(t1[:], kq["f"][:], t1[:])   # f - Kx*uxf (flipped)
        nc.vector.tensor_mul(t2[:], kq["Ky"][:], fin["uy"][:])
        nc.vector.tensor_sub(t1[:], t1[:], t2[:])
        nc.vector.tensor_mul(t2[:], kq["K"][:], fin["S"][:])
        nc.vector.tensor_add(pde[:], t1[:], t2[:])
        nc.sync.dma_start(
            out=pde_d[:].rearrange("(p j) -> p j", p=128), in_=pde[:]
        )

    if not nc.is_finalized():
        nc.finalize()
    legalized = _legalize_sync_waits(nc.to_json_bytes())
    nc.to_json_bytes = lambda: legalized
    return nc


_NC = None


def _get_nc():
    global _NC
    if _NC is None:
        _patch_walrus_ldw_opt()
        _NC = build_nc()
    return _NC


def make_in_maps(inputs):
    """Shard full inputs into per-core input maps."""
    full = {k: np.asarray(v, dtype=np.float32) for k, v in inputs.items()}
    in_maps = []
    for c in range(NCORES):
        s = slice(c * BC, (c + 1) * BC)
        m = {
            "xy": full["xy"][s],
            "K": full["K"][s],
            "Kx": full["Kx"][s],
            "Ky": full["Ky"][s],
            "f": full["f"][s],
            "Wout": full["Wout"],
            "bout": full["bout"].reshape(1),
        }
        for i in range(L):
            m[f"W{i}"] = full[f"W{i}"]
            m[f"b{i}"] = full[f"b{i}"]
        in_maps.append(m)
    return in_maps


def run(inputs, trace=False, **kw):
    from concourse.bass_utils import run_bass_kernel_spmd

    nc = _get_nc()
    res = run_bass_kernel_spmd(
        nc, make_in_maps(inputs), list(range(NCORES)), trace=trace, **kw
    )
    u = np.concatenate([r["u"] for r in res.results])
    pde = np.concatenate([r["pde"] for r in res.results])
    return (u, pde), res


def kernel(**inputs):
    (u, pde), _ = run(inputs)
    return u, pde



# revision 13
# speedup vs baseline: 1.1447x; 1.1447x over previous
"""Trainium2 Bass kernel for MembraneNet (PINN forward + analytic PDE residual).

Math: per collocation point p=(x,y):
  u(p)  = Wout . h3 + bout           (4-layer tanh MLP, H=64)
  PDE   = K*(uxx+uyy) + Kx*ux + Ky*uy + f
Forward-mode propagation of (h, gx, gy, lap) per layer:
  z  = W h + b ;  h' = tanh(z) ;  d = 1-h'^2 ;  s = -2 h' d
  zx = W gx    ;  gx' = d . zx      (stored as dbar.zx = -d.zx -> sign
  zy = W gy    ;  gy' = d . zy       alternates per layer; after 4 layers
  zl = W lap   ;  lap' = s.(zx^2+zy^2) + d.zl      it is back to true sign)
lap is carried unsummed as [dd, ee]; the next layer's zl matmul accumulates
both in PSUM.

Layout: batch 16384 split over 8 cores (2048/core); on each core 2 chunks of
1024 points stacked on SBUF partitions 0-63 / 64-127 with block-diagonal
weights so matmuls use the full 128-wide PE array.

Perf design vs the f32 baseline:
- fp16 everywhere on-chip (PE streams 16-bit rhs at 1 col/cyc vs ~3 for f32;
  DVE tensor_tensor gets 2x mode, tensor_scalar 4x).
- Elementwise split across Act (tanh, squares, PSUM->SBUF moves) and DVE
  (tensor_tensor chains), both ~5us/layer.
- Host packs weights/biases/coords into few tensors -> 13 input DMAs spread
  over all 5 engine queues (was ~27 serialized on 2).
- One fused [8,F] reduction PSUM tile (u/ux/uy/S) -> single 4D reshape DMA.
"""

import sys

sys.path.insert(0, "/opt/trn_rl_repo")

import numpy as np
from contextlib import ExitStack

import concourse.bass as bass
import concourse.mybir as mybir
import concourse.tile as tile
from concourse.masks import make_identity

B = 16384
H = 64
L = 4
NCORES = 8
BC = B // NCORES          # 2048 points per core
F = BC // 2               # 1024 free-dim columns (2 chunks stacked)
NH = 512                  # matmul free-dim per instruction (1 PSUM bank fp32)
NMM = F // NH
FT = BC // 128            # 16: free dim of final per-point [128, FT] tiles

f32 = mybir.dt.float32
f16 = mybir.dt.float16
AF = mybir.ActivationFunctionType
OP = mybir.AluOpType

MM_DT = f16               # matmul operand dtype
EW_DT = f16               # elementwise dtype

WB_BIAS = 3 * H * H       # offset of the 8x128 bias block in wb


def _patch_walrus_ldw_opt():
    """Re-enable walrus LDWEIGHTS dedup (concourse pins it off)."""
    from concourse import bass_utils as _bu

    if getattr(_bu, "_ldw_opt_patched", False):
        return
    orig = _bu.run_command

    def patched(argv, **kw):
        argv = [
            "--enable-ldw-opt=true" if a == "--enable-ldw-opt=false" else a
            for a in argv
        ]
        return orig(argv, **kw)

    _bu.run_command = patched
    _bu._ldw_opt_patched = True


def _legalize_sync_waits(bj: bytes) -> bytes:
    """The walrus in this container accepts at most ONE on_wait per
    instruction, but Tile emits several. Move excess waits into standalone
    EventSemaphore instructions right before the owner (same engine, so the
    sequencer executes them first)."""
    import json

    m = json.loads(bj)
    n = 0
    for fn in m.get("functions", []):
        for blk in fn.get("blocks", []):
            out = []
            for ins in blk.get("instructions", []):
                si = ins.get("sync_info") or {}
                waits = si.get("on_wait") or []
                if len(waits) > 1:
                    for w in waits[:-1]:
                        n += 1
                        out.append(
                            {
                                "name": f"lsw_{n}",
                                "opcode": "EventSemaphore",
                                "engine": ins["engine"],
                                "ins": [],
                                "outs": [],
                                "debug": ins.get("debug", 0),
                                "sync_info": {"on_update": [], "on_wait": [w]},
                            }
                        )
                    si["on_wait"] = waits[-1:]
                out.append(ins)
            blk["instructions"] = out
    return json.dumps(m).encode()


def build_nc(mm_dt=MM_DT, ew_dt=EW_DT):
    nc = bass.Bass()

    # ---- I/O (host-packed; see make_in_maps) ----
    xc_d = nc.dram_tensor("xc", [BC], f16, kind="ExternalInput")
    yc_d = nc.dram_tensor("yc", [BC], f16, kind="ExternalInput")
    w0r_d = nc.dram_tensor("w0r", [2 * H], f16, kind="ExternalInput")
    wb_d = nc.dram_tensor("wb", [WB_BIAS + 8 * 128], f32, kind="ExternalInput")
    kkf_d = nc.dram_tensor("kkf", [4 * BC], f32, kind="ExternalInput")
    u_d = nc.dram_tensor("u", [BC], f32, kind="ExternalOutput")
    pde_d = nc.dram_tensor("pde", [BC], f32, kind="ExternalOutput")

    with tile.TileContext(nc) as tc, ExitStack() as ctx:
        const = ctx.enter_context(tc.tile_pool(name="const", bufs=1))
        sb = ctx.enter_context(tc.tile_pool(name="sb", bufs=2))
        ps = ctx.enter_context(tc.tile_pool(name="ps", bufs=1, space="PSUM"))

        # ---- memsets for zero-padded tiles ----
        xyT = const.tile([128, F], mm_dt, tag="xyT")    # L0 rhs
        wt0 = const.tile([128, 128], mm_dt, tag="wt0")  # L0 lhsT
        wl32f = const.tile([128, 32], f32, tag="wl32f")
        nc.vector.memset(xyT[:], 0.0)
        nc.vector.memset(wt0[:], 0.0)
        nc.vector.memset(wl32f[:], 0.0)
        ident = const.tile([128, 128], f32, tag="ident")
        make_identity(nc, ident[:])
        wnat = []
        for k in range(1, L):
            w = const.tile([128, 128], f32, tag=f"wnat{k}")
            nc.gpsimd.memset(w[:], 0.0)
            wnat.append(w)

        # ---- input DMAs, spread across the 5 engine queues.
        # L0-critical first: xyT rows {0,32,64,96}, wt0 rows, bias block. ----
        nc.sync.dma_start(out=xyT[0:1, :], in_=xc_d[0:F][None, :])
        nc.sync.dma_start(out=xyT[H : H + 1, :], in_=xc_d[F:BC][None, :])
        nc.scalar.dma_start(out=xyT[32:33, :], in_=yc_d[0:F][None, :])
        nc.scalar.dma_start(out=xyT[96:97, :], in_=yc_d[F:BC][None, :])

        stg2 = const.tile([128, 128], f32, tag="stg2")
        nc.gpsimd.dma_start(
            out=stg2[0:8, :],
            in_=wb_d[WB_BIAS : WB_BIAS + 1024].rearrange("(p c) -> p c", p=8),
        )
        nc.gpsimd.dma_start(out=wt0[0:1, 0:H], in_=w0r_d[0:H][None, :])
        nc.gpsimd.dma_start(
            out=wt0[32:33, 0:H], in_=w0r_d[H : 2 * H][None, :]
        )
        nc.gpsimd.dma_start(
            out=wt0[H : H + 1, H:128], in_=w0r_d[0:H][None, :]
        )
        nc.gpsimd.dma_start(
            out=wt0[96:97, H:128], in_=w0r_d[H : 2 * H][None, :]
        )

        # dummy tanh: triggers ACT_TABLE_LOAD here so the ~2.7us table load
        # overlaps the remaining preamble instead of stalling the first tanh
        ztiny = const.tile([1, 2], f32, tag="ztiny")
        nc.vector.memset(ztiny[:], 0.0)
        dum = const.tile([1, 2], ew_dt, tag="dum")
        nc.scalar.activation(dum[:], ztiny[:], AF.Tanh)

        # weights W1..W3: block-diag staging (2 DMAs each)
        for i, k in enumerate(range(1, L)):
            wk = wb_d[(k - 1) * H * H : k * H * H].rearrange(
                "(p c) -> p c", p=H
            )
            nc.sync.dma_start(out=wnat[i][0:H, 0:H], in_=wk)
            nc.scalar.dma_start(out=wnat[i][H:128, H:128], in_=wk)

        # per-point Kx|Ky|K|f tile [128, 4*FT]
        kkft = const.tile([128, 4 * FT], f32, tag="kkft")
        nc.gpsimd.dma_start(
            out=kkft[:].rearrange("p (q k) -> p q k", q=4),
            in_=kkf_d[:].rearrange("(q p k) -> p q k", q=4, p=128, k=FT),
        )

        # ---- bias/Wout/W0 columns via one transpose: bwc col j =
        # [b0..b3, Wout, W0x, W0y, bout] ----
        bw_ps = ps.tile([128, 128], f32, tag="zxp")
        nc.tensor.transpose(bw_ps[:], stg2[:], ident[:])
        bwc = const.tile([128, 8], f32, tag="bwc")
        nc.vector.tensor_copy(bwc[:], bw_ps[:, 0:8])
        bcol = [bwc[:, k : k + 1] for k in range(L)]

        # ---- weight transposes -> fp16 block-diag lhsT ----
        WT = [wt0]
        pstags = ["zp", "zyp", "zlp"]
        for i, k in enumerate(range(1, L)):
            wt_ps = ps.tile([128, 128], f32, tag=pstags[i])
            nc.tensor.transpose(wt_ps[:], wnat[i][:], ident[:])
            wt = const.tile([128, 128], mm_dt, tag=f"wt{k}")
            nc.scalar.copy(wt[:], wt_ps[:])
            WT.append(wt)

        # ---- output-reduction lhsT: group g has local col g = [Wout;0]
        # (row g, chunk A) and local col g+4 = [0;Wout] (row g+4, chunk B) ----
        for g in range(4):
            nc.vector.tensor_copy(wl32f[0:H, 9 * g : 9 * g + 1], bwc[0:H, 4:5])
            nc.vector.tensor_copy(
                wl32f[H:128, 9 * g + 4 : 9 * g + 5], bwc[H:128, 4:5]
            )
        wl32 = const.tile([128, 32], mm_dt, tag="wl32")
        nc.vector.tensor_copy(wl32[:], wl32f[:])
        wout_l = [wl32[:, 8 * g : 8 * g + 8] for g in range(4)]

        # q0p1 = W0x^2 + W0y^2 per partition (layer-0 laplacian source)
        w5sq = const.tile([128, 1], f32, tag="w5sq")
        w6sq = const.tile([128, 1], f32, tag="w6sq")
        q0p1 = const.tile([128, 1], f32, tag="q0p1")
        nc.vector.tensor_mul(w5sq[:], bwc[:, 5:6], bwc[:, 5:6])
        nc.vector.tensor_mul(w6sq[:], bwc[:, 6:7], bwc[:, 6:7])
        nc.vector.tensor_add(q0p1[:], w5sq[:], w6sq[:])

        def mm_pass(out_ps, lhsT, rhs, start=True, stop=True):
            for j in range(NMM):
                s = slice(j * NH, (j + 1) * NH)
                nc.tensor.matmul(
                    out_ps[:, s], lhsT[:], rhs[:, s], start=start, stop=stop
                )

        # ---- layer 0 ----
        zp = ps.tile([128, F], f32, tag="zp")
        mm_pass(zp, WT[0], xyT)
        h = sb.tile([128, F], mm_dt, tag="h")
        hsq = sb.tile([128, F], ew_dt, tag="hsq")
        nc.scalar.activation(h[:], zp[:], AF.Tanh, bias=bcol[0], scale=1.0)
        nc.scalar.activation(hsq[:], h[:], AF.Square)
        dbar = sb.tile([128, F], ew_dt, tag="dbar")
        gx = sb.tile([128, F], mm_dt, tag="gx")
        gy = sb.tile([128, F], mm_dt, tag="gy")
        m2 = sb.tile([128, F], ew_dt, tag="m2")
        lap0 = sb.tile([128, F], mm_dt, tag="dd")
        nc.vector.tensor_scalar_add(dbar[:], hsq[:], -1.0)
        nc.vector.tensor_scalar_mul(gx[:], dbar[:], bwc[:, 5:6])
        nc.vector.tensor_scalar_mul(gy[:], dbar[:], bwc[:, 6:7])
        nc.vector.scalar_tensor_tensor(
            m2[:], h[:], 2.0, dbar[:], OP.mult, OP.mult
        )  # = s
        nc.vector.tensor_scalar_mul(lap0[:], m2[:], q0p1[:])  # s * q0
        lsrc = [lap0]

        # ---- layers 1..3 ----
        for k in range(1, L):
            zp = ps.tile([128, F], f32, tag="zp")
            zxp = ps.tile([128, F], f32, tag="zxp")
            zyp = ps.tile([128, F], f32, tag="zyp")
            zlp = ps.tile([128, F], f32, tag="zlp")
            mm_pass(zp, WT[k], h)
            mm_pass(zxp, WT[k], gx)
            mm_pass(zyp, WT[k], gy)
            for i, ls in enumerate(lsrc):
                mm_pass(
                    zlp, WT[k], ls, start=(i == 0), stop=(i == len(lsrc) - 1)
                )

            h = sb.tile([128, F], mm_dt, tag="h")
            hsq = sb.tile([128, F], ew_dt, tag="hsq")
            zxsq = sb.tile([128, F], ew_dt, tag="zxsq")
            zysq = sb.tile([128, F], ew_dt, tag="zysq")
            zlc = sb.tile([128, F], ew_dt, tag="zlc")
            nc.scalar.activation(h[:], zp[:], AF.Tanh, bias=bcol[k], scale=1.0)
            nc.scalar.activation(hsq[:], h[:], AF.Square)
            nc.scalar.activation(zxsq[:], zxp[:], AF.Square)
            nc.scalar.activation(zysq[:], zyp[:], AF.Square)
            nc.scalar.copy(zlc[:], zlp[:])

            dbar = sb.tile([128, F], ew_dt, tag="dbar")
            gx = sb.tile([128, F], mm_dt, tag="gx")
            gy = sb.tile([128, F], mm_dt, tag="gy")
            q = sb.tile([128, F], ew_dt, tag="q")
            t = sb.tile([128, F], ew_dt, tag="m2")
            dd = sb.tile([128, F], mm_dt, tag="dd")
            ee = sb.tile([128, F], mm_dt, tag="ee")
            nc.vector.tensor_scalar_add(dbar[:], hsq[:], -1.0)
            nc.vector.tensor_mul(gx[:], dbar[:], zxp[:])
            nc.vector.tensor_mul(gy[:], dbar[:], zyp[:])
            nc.vector.tensor_add(q[:], zxsq[:], zysq[:])
            nc.vector.tensor_mul(t[:], dbar[:], q[:])
            nc.vector.scalar_tensor_tensor(
                dd[:], t[:], 2.0, h[:], OP.mult, OP.mult
            )  # s*q
            nc.vector.scalar_tensor_tensor(
                ee[:], zlc[:], -1.0, dbar[:], OP.mult, OP.mult
            )  # d*zl
            lsrc = [dd, ee]

        # ---- fused output reduction: one [8, F] PSUM tile.
        # group g -> rows (g, g+4): g0=ux, g1=uy, g2=S, g3=u ----
        rp = ps.tile([8, F], f32, tag="zxp")
        srcs = [(3, h), (0, gx), (1, gy), (2, dd), (2, ee)]
        for i, (g, src) in enumerate(srcs):
            for j in range(NMM):
                s = slice(j * NH, (j + 1) * NH)
                nc.tensor.matmul(
                    rp[:, s], wout_l[g], src[:, s],
                    start=(i == 0), stop=(i == len(srcs) - 1),
                )

        # ---- reshape rows -> per-point [128, 4*FT] tile via one DMA
        # T cols: ux 0:16 | uy 16:32 | S 32:48 | u 48:64 ----
        red = sb.tile([8, F], f32, tag="red")
        nc.scalar.copy(red[:], rp[:])
        T = sb.tile([128, 4 * FT], f32, tag="T")
        engs = [nc.sync, nc.scalar, nc.gpsimd]
        for i, (hh, q) in enumerate(
            (hh, q) for q in range(4) for hh in range(2)
        ):
            engs[i % 3].dma_start(
                out=T[hh * H : (hh + 1) * H, q * FT : (q + 1) * FT],
                in_=red[4 * hh + q : 4 * hh + q + 1, :],
            )

        # ---- final assembly ----
        m = sb.tile([128, 3 * FT], f32, tag="m")
        p1 = sb.tile([128, FT], f32, tag="p1")
        p2 = sb.tile([128, FT], f32, tag="p2")
        pde = sb.tile([128, FT], f32, tag="pde")
        u_fin = sb.tile([128, FT], f32, tag="u_fin")
        nc.vector.tensor_mul(m[:], T[:, 0 : 3 * FT], kkft[:, 0 : 3 * FT])
        nc.vector.tensor_add(p1[:], m[:, 0:FT], m[:, FT : 2 * FT])
        nc.vector.tensor_add(
            p2[:], m[:, 2 * FT : 3 * FT], kkft[:, 3 * FT : 4 * FT]
        )
        nc.vector.tensor_add(pde[:], p1[:], p2[:])
        nc.vector.tensor_scalar_add(u_fin[:], T[:, 3 * FT : 4 * FT], bwc[:, 7:8])
        nc.sync.dma_start(
            out=pde_d[:].rearrange("(p j) -> p j", p=128), in_=pde[:]
        )
        nc.scalar.dma_start(
            out=u_d[:].rearrange("(p j) -> p j", p=128), in_=u_fin[:]
        )

    if not nc.is_finalized():
        nc.finalize()
    legalized = _legalize_sync_waits(nc.to_json_bytes())
    nc.to_json_bytes = lambda: legalized
    return nc


_NC = None


def _get_nc():
    global _NC
    if _NC is None:
        # NOTE: walrus LDW-opt is left OFF — it rejects fp16 LDWEIGHTS.
        _NC = build_nc()
    return _NC


def make_in_maps(inputs):
    """Shard full inputs into per-core input maps (host-side packing)."""
    full = {k: np.asarray(v, dtype=np.float32) for k, v in inputs.items()}
    W0 = full["W0"]
    w0r = np.concatenate([W0[:, 0], W0[:, 1]]).astype(np.float16)
    bias_block = np.zeros((8, 128), np.float32)
    for i in range(L):
        bias_block[i] = np.tile(full[f"b{i}"], 2)
    bias_block[4] = np.tile(full["Wout"], 2)
    bias_block[5] = np.tile(W0[:, 0], 2)
    bias_block[6] = np.tile(W0[:, 1], 2)
    bias_block[7] = full["bout"]
    wb = np.concatenate(
        [
            full["W1"].ravel(),
            full["W2"].ravel(),
            full["W3"].ravel(),
            bias_block.ravel(),
        ]
    ).astype(np.float32)

    in_maps = []
    for c in range(NCORES):
        s = slice(c * BC, (c + 1) * BC)
        xy = full["xy"][s]
        kkf = np.concatenate(
            [full["Kx"][s], full["Ky"][s], full["K"][s], full["f"][s]]
        ).astype(np.float32)
        in_maps.append(
            {
                "xc": np.ascontiguousarray(xy[:, 0]).astype(np.float16),
                "yc": np.ascontiguousarray(xy[:, 1]).astype(np.float16),
                "w0r": w0r,
                "wb": wb,
                "kkf": kkf,
            }
        )
    return in_maps


def run(inputs, trace=False, **kw):
    from concourse.bass_utils import run_bass_kernel_spmd

    nc = _get_nc()
    res = run_bass_kernel_spmd(
        nc, make_in_maps(inputs), list(range(NCORES)), trace=trace, **kw
    )
    u = np.concatenate([r["u"] for r in res.results])
    pde = np.concatenate([r["pde"] for r in res.results])
    return (u, pde), res


def kernel(**inputs):
    (u, pde), _ = run(inputs)
    return u, pde
